# revision 27
# baseline (speedup 1.0000x reference)
"""MeanField CRF message-passing kernel for 8 Trainium2 NeuronCores.

Sharding: (B=2) x (H into 4 chunks of 128 rows) = 8 slabs, each with a
5-row halo on slab-interior edges (5 mean-field iterations x 1-row
stencil reach), so cores run fully independently (no collectives).

Per-core layouts (all 16-bit except fp32 softmax sums / PSUM):
  x-layout  : [x mod 128 -> partitions, (xblock, class, y) -> free]
  C-packed  : [(class*6+row)=126 (+2 pad) -> partitions, x -> free]
Math per iteration (equivalent-transformed from the reference):
  E   = exp(-Y)                        (ACT, bf16, class-major blocks)
  s   = sum_c E ; r = 1/s              (DVE reduce f32 + recip -> bf16)
  m   = blockdiag(LC^T/8) @ E_C        (PE matmul; E_C via XBAR DMA transpose)
  w2_d= ew_d * shift_d(r)              (DVE bf16 2x, normalizer folded)
  t_d = w2_d * shift_d(m)              (DVE bf16 2x muls on XBAR-transposed m)
  Y   = u + sum_d t_d                  (PE identity-matmuls accumulate in PSUM
                                        with per-dy shifted out-APs; ACT evac)
Window boundary rows (2 per seam) are evacuated as partials and combined
with the next window's PSUM by one tiny DVE add per seam.
Final cost = Y after iteration 5 (no softmax on the last iteration).
"""

import sys

sys.path.insert(0, "/opt/trn_rl_repo")

import numpy as np
import ml_dtypes

import concourse.bass as bass
import concourse.bacc as bacc
import concourse.tile as tile
from concourse import mybir
from concourse.bass_utils import run_bass_kernel_spmd

F32 = mybir.dt.float32
F16 = mybir.dt.float16
BF16 = mybir.dt.bfloat16

P = 128          # partitions
C = 21           # classes
RG = 6           # y-rows per packed group (21*6=126 of 128 partitions)
PK = 128         # padded packed-block size
NB = 23          # row-groups per slab (138 = 6*23)
YT = 138         # slab rows (128 own + 2*5 halo)
YTP = 140        # padded rows (1 pad row each end)
XB = 4           # x blocks (512 = 4*128)
D = 8            # directions
W = 512
HALO = 5
OWN = 128
MAX_ITER = 5
DIRS = [(0, 1), (0, -1), (1, 0), (-1, 0), (1, 1), (1, -1), (-1, 1), (-1, -1)]
# (dy, [(dir index, dx), ...])
GROUPS = [
    (0, [(0, 1), (1, -1)]),
    (1, [(2, 0), (4, 1), (5, -1)]),
    (-1, [(3, 0), (6, 1), (7, -1)]),
]
WINDOWS = [(0, 6), (6, 6), (12, 6), (18, 5)]  # (first group, n groups)
ZN = 38          # PSUM window rows (36 own + 2 boundary)
CA = 13          # class-split sizes (matmul out <= 512 f32 = 1 PSUM bank)
CB = C - CA

_CACHED_NC = None

ADD = mybir.AluOpType.add


def build_nc():
    nc = bacc.Bacc("TRN2")
    uu_d = nc.dram_tensor("uu", [P, XB, C, YTP], BF16, kind="ExternalInput")
    ew_d = nc.dram_tensor("ew", [P, D, XB, YT], F16, kind="ExternalInput")
    lcb_d = nc.dram_tensor("lcblk", [C * RG, C * RG], BF16, kind="ExternalInput")
    ide_d = nc.dram_tensor("ident", [P, P], BF16, kind="ExternalInput")
    yout_d = nc.dram_tensor("yout", [P, XB, C, YTP], BF16, kind="ExternalOutput")

    with tile.TileContext(nc) as tc:
        with (
            tc.tile_pool(name="state", bufs=1) as st,
            tc.tile_pool(name="mxp", bufs=2) as mxp,
            tc.tile_pool(name="tp", bufs=12) as tp,
            tc.tile_pool(name="sbp", bufs=2) as sbp,
            tc.tile_pool(name="trp", bufs=2) as trp,
            tc.tile_pool(name="pm", bufs=3, space="PSUM") as pm,
            tc.tile_pool(name="pw", bufs=2, space="PSUM") as pw,
        ):
            UU = st.tile([P, XB, C, YTP], BF16)
            Y = st.tile([P, XB, C, YTP], BF16)
            EB = st.tile([P, XB, NB, PK], BF16)   # exp(-Y), class-major blocks
            EC = st.tile([P, XB, NB, PK], BF16)   # transposed E (C-packed)
            MC = st.tile([P, PK + NB * W + PK], BF16)  # m, flat, C-packed
            EWs = st.tile([P, D, XB, YT], F16)
            W2P = st.tile([P, D, XB, YTP], BF16)
            S0 = st.tile([P, YT, XB], BF16)
            S0B = st.tile([P, YT, XB], BF16)
            RP = st.tile([P, YT, XB], BF16)
            RM = st.tile([P, YT, XB], BF16)
            LCB = st.tile([C * RG, C * RG], BF16)
            IDE = st.tile([P, P], BF16)
            ZR2 = st.tile([P, C * 2], BF16)

            nc.sync.dma_start(out=UU[:], in_=uu_d[:])
            nc.sync.dma_start(out=EWs[:], in_=ew_d[:])
            nc.sync.dma_start(out=LCB[:], in_=lcb_d[:])
            nc.sync.dma_start(out=IDE[:], in_=ide_d[:])
            nc.gpsimd.memset(Y[:], 0)
            nc.gpsimd.memset(EB[:], 0)
            nc.gpsimd.memset(MC[:], 0)
            nc.gpsimd.memset(W2P[:], 0)
            nc.gpsimd.memset(RP[:], 0)
            nc.gpsimd.memset(RM[:], 0)
            nc.vector.memset(ZR2[:], 0)

            def emit_exp(it, b0, b1):
                srcT = UU if it == 0 else Y
                for xb in range(XB):
                    ev = EB[:, xb, b0:b1, 0:126].rearrange(
                        "p b (c r) -> p c b r", c=C, r=RG
                    )
                    yv = srcT[:, xb, :, 1 + b0 * RG : 1 + b1 * RG].rearrange(
                        "p c (b r) -> p c b r", b=b1 - b0, r=RG
                    )
                    nc.scalar.activation(
                        out=ev, in_=yv,
                        func=mybir.ActivationFunctionType.Exp, scale=-1.0,
                    )

            def emit_tree(b0, b1):
                nb = b1 - b0
                for xb in range(XB):
                    ebv = EB[:, xb, b0:b1, 0:126].rearrange(
                        "p b (c r) -> p b c r", c=C, r=RG
                    )
                    tr = trp.tile([P, NB, 11, RG], BF16, tag="tr")
                    tv = tr[:, b0:b1]
                    nc.vector.tensor_add(
                        out=tv[:, :, 0:10, :], in0=ebv[:, :, 0:10, :],
                        in1=ebv[:, :, 10:20, :],
                    )
                    nc.vector.tensor_add(
                        out=tv[:, :, 0:5, :], in0=tv[:, :, 0:5, :],
                        in1=tv[:, :, 5:10, :],
                    )
                    nc.vector.tensor_add(
                        out=tv[:, :, 0:1, :], in0=tv[:, :, 0:1, :],
                        in1=ebv[:, :, 20:21, :],
                    )
                    nc.vector.tensor_add(
                        out=tv[:, :, 0:2, :], in0=tv[:, :, 0:2, :],
                        in1=tv[:, :, 2:4, :],
                    )
                    nc.vector.tensor_add(
                        out=tv[:, :, 0:1, :], in0=tv[:, :, 0:1, :],
                        in1=tv[:, :, 1:2, :],
                    )
                    nc.vector.tensor_add(
                        out=S0[:, b0 * RG : b1 * RG, xb].rearrange(
                            "p (b r) -> p b r", b=nb, r=RG
                        ),
                        in0=tv[:, :, 0, :],
                        in1=tv[:, :, 4, :],
                    )

            def emit_recip(r0, r1):
                with nc.allow_low_precision(reason="r scales bf16 q"):
                    nc.vector.reciprocal(
                        out=S0B[:, r0:r1, :], in_=S0[:, r0:r1, :]
                    )

            def emit_shift(r0, r1):
                nc.gpsimd.dma_start(
                    out=RP[0 : P - 1, r0:r1, :], in_=S0B[1:P, r0:r1, :]
                )
                nc.gpsimd.dma_start(
                    out=RP[P - 1 : P, r0:r1, 0 : XB - 1],
                    in_=S0B[0:1, r0:r1, 1:XB],
                )
                nc.gpsimd.dma_start(
                    out=RM[1:P, r0:r1, :], in_=S0B[0 : P - 1, r0:r1, :]
                )
                nc.gpsimd.dma_start(
                    out=RM[0:1, r0:r1, 1:XB],
                    in_=S0B[P - 1 : P, r0:r1, 0 : XB - 1],
                )

            def emit_w2(r0, r1):
                for d, (dy, dx) in enumerate(DIRS):
                    rsrc = {-1: RM, 0: S0B, 1: RP}[dx]
                    lo = max(max(0, -dy), r0)
                    hi = min(min(YT, YT - dy), r1)
                    if hi <= lo:
                        continue
                    for xb in range(XB):
                        nc.gpsimd.tensor_mul(
                            out=W2P[:, d, xb, 1 + lo : 1 + hi],
                            in0=EWs[:, d, xb, lo:hi],
                            in1=rsrc[:, lo + dy : hi + dy, xb],
                        )

            def emit_ec(b0, b1):
                for xb in range(XB):
                    nc.sync.dma_start_transpose(
                        out=EC[:, xb, b0:b1, :],
                        in_=EB[:, xb, b0:b1, :].rearrange("p a b -> p (a b)"),
                    )

            BLO = 7            # hoisted prefix: blocks [0,7) -> rows [0,42)
            RLO = BLO * RG
            for it in range(MAX_ITER):
                # ---- phase A head (lo part hoisted into prior iteration) -
                if it == 0:
                    emit_exp(it, 0, BLO)
                    emit_tree(0, BLO)
                    emit_recip(0, RLO)
                    emit_shift(0, RLO)
                    emit_w2(0, RLO - 3)
                    emit_ec(0, BLO)
                emit_exp(it, BLO, NB)
                emit_tree(BLO, NB)
                emit_recip(RLO, YT)
                emit_shift(RLO, YT)
                emit_w2(RLO - 3, YT)
                if it == 0:
                    emit_ec(BLO, NB)

                def emit_mm(rb0, rb1):
                    for rb in range(rb0, rb1):
                        mcp = pm.tile([126, W], F32, tag="mcp")
                        nc.tensor.matmul(
                            out=mcp[:],
                            lhsT=LCB[:],
                            rhs=EC[0:126, :, rb, :],
                            start=True, stop=True,
                        )
                        nc.scalar.copy(
                            out=MC[0:126, PK + rb * W : PK + (rb + 1) * W],
                            in_=mcp[:],
                        )

                emit_mm(0, BLO)
                # pre-emit window 0's shift transposes so they run as soon
                # as the first blocks are evacuated; EC_hi and the remaining
                # matmuls are emitted after (and so execute after) them.
                mxt0 = {}
                for dx in (1, -1, 0):
                    t_mx = mxp.tile([P, 6, XB, PK], BF16, tag=f"mx{dx + 1}")
                    mxt0[dx] = t_mx
                    nc.sync.dma_start_transpose(
                        out=t_mx[:, 0:6, :, :],
                        in_=MC[:, PK + dx : PK + dx + 6 * W],
                    )
                if it > 0:
                    emit_ec(BLO, 13)
                emit_mm(BLO, 13)

                # ---- phase B: XBAR shift transposes, DVE muls, PE accum --
                sbt_prev = {}
                for w, (g0, ng) in enumerate(WINDOWS):
                    zw = g0 * RG            # window base row (pad coords)
                    n = ng * RG             # own rows in window
                    zn = n + 2              # PSUM rows incl. boundary pair
                    last = w == len(WINDOWS) - 1
                    if w == 0:
                        mxt = mxt0
                    else:
                        mxt = {}
                        for dx in (1, -1, 0):
                            t_mx = mxp.tile(
                                [P, 6, XB, PK], BF16, tag=f"mx{dx + 1}"
                            )
                            mxt[dx] = t_mx
                            a0 = PK + g0 * W + dx
                            nc.sync.dma_start_transpose(
                                out=t_mx[:, 0:ng, :, :],
                                in_=MC[:, a0 : a0 + ng * W],
                            )
                    for xb in range(XB):
                        # -- DVE: 8 products t_d = w2_d * m_shift ----------
                        ts = []
                        for dy, dirs_g in GROUPS:
                            z0 = zw - dy + 1
                            for d, dx in dirs_g:
                                t = tp.tile([P, C, 36], BF16, tag="tt")
                                tv = t[:, :, 0 : ng * RG].rearrange(
                                    "p c (g r) -> p g c r", g=ng, r=RG
                                )
                                mxv = mxt[dx][:, 0:ng, xb, 0:126].rearrange(
                                    "p g (c r) -> p g c r", c=C, r=RG
                                )
                                w2v = (
                                    W2P[:, d, xb, z0 : z0 + n]
                                    .rearrange(
                                        "p (g c r) -> p g c r", g=ng, c=1, r=RG
                                    )
                                    .to_broadcast((P, ng, C, RG))
                                )
                                nc.vector.tensor_mul(out=tv, in0=mxv, in1=w2v)
                                ts.append((dy, t))
                        # -- PE: accumulate u + 8 terms in PSUM ------------
                        # matmul out is limited to one PSUM bank (512 f32),
                        # so the [C, ZN] window is split by class range.
                        ywA = pw.tile([P, CA, ZN], F32, tag="ywA")
                        ywB = pw.tile([P, CB, ZN], F32, tag="ywB")
                        halves = ((ywA, 0, CA), (ywB, CA, CB))
                        for yw, c0, cn in halves:
                            if w == 0:
                                nc.tensor.matmul(
                                    out=yw[:, :, 0:zn],
                                    lhsT=IDE[:],
                                    rhs=UU[:, xb, c0 : c0 + cn, zw : zw + zn],
                                    start=True, stop=False,
                                )
                            else:
                                nc.tensor.matmul(
                                    out=yw[:, :, 0:2],
                                    lhsT=IDE[:],
                                    rhs=ZR2[:, 0 : cn * 2].rearrange(
                                        "p (c z) -> p c z", c=cn
                                    ),
                                    start=True, stop=False,
                                )
                                nc.tensor.matmul(
                                    out=yw[:, :, 2:zn],
                                    lhsT=IDE[:],
                                    rhs=UU[:, xb, c0 : c0 + cn, zw + 2 : zw + zn],
                                    start=True, stop=False,
                                )
                            for i, (dy, t) in enumerate(ts):
                                zb = 1 - dy
                                nc.tensor.matmul(
                                    out=yw[:, :, zb : zb + n],
                                    lhsT=IDE[:],
                                    rhs=t[:, c0 : c0 + cn, 0 : ng * RG],
                                    start=False, stop=(i == len(ts) - 1),
                                )
                        # -- evacuate + seam handling ----------------------
                        ea = 0 if w == 0 else 2
                        eb = zn if last else n
                        if not last:
                            sbt = sbp.tile([P, C, 2], F32, tag=f"sb{xb}")
                        for yw, c0, cn in halves:
                            nc.scalar.copy(
                                out=Y[:, xb, c0 : c0 + cn, zw + ea : zw + eb],
                                in_=yw[:, :, ea:eb],
                            )
                            if not last:
                                nc.scalar.copy(
                                    out=sbt[:, c0 : c0 + cn, :],
                                    in_=yw[:, :, n : n + 2],
                                )
                            if w > 0:
                                nc.vector.tensor_add(
                                    out=Y[:, xb, c0 : c0 + cn, zw : zw + 2],
                                    in0=sbt_prev[xb][:, c0 : c0 + cn, :],
                                    in1=yw[:, :, 0:2],
                                )
                        if not last:
                            sbt_prev[xb] = sbt
                    if w == 0:
                        if it > 0:
                            emit_ec(13, NB)
                        emit_mm(13, NB)
                    if w == 1 and it < MAX_ITER - 1:
                        # hoist next iteration's low-row pipeline: rows
                        # [0,42) of Y are final after window 1's evac+seam.
                        emit_exp(it + 1, 0, BLO)
                        emit_tree(0, BLO)
                        emit_recip(0, RLO)
                        emit_shift(0, RLO)
                        emit_w2(0, RLO - 3)
                        emit_ec(0, BLO)

            nc.sync.dma_start(out=yout_d[:], in_=Y[:])

    nc.finalize()
    return nc


def _prep_core(u, ew, b, hc):
    y0 = 128 * hc
    ys = min(max(y0 - HALO, 0), 512 - YT)
    u_slab = u[b, 0, :, ys : ys + YT, :]          # [21, 138, 512]
    ew_slab = ew[b, :, ys : ys + YT, :]           # [8, 138, 512]
    uu = np.zeros((P, XB, C, YTP), np.float32)
    uu[:, :, :, 1 : 1 + YT] = u_slab.reshape(C, YT, XB, P).transpose(3, 2, 0, 1)
    ewp = np.ascontiguousarray(
        ew_slab.reshape(D, YT, XB, P).transpose(3, 0, 2, 1), dtype=np.float16
    )
    return uu.astype(ml_dtypes.bfloat16), ewp, ys, y0 - ys


def kernel(unary, edge_weights, label_context, _trace=False, _tmpdir=None):
    global _CACHED_NC
    if _CACHED_NC is None:
        _CACHED_NC = build_nc()
    nc = _CACHED_NC

    u = np.asarray(unary, dtype=np.float32)
    ew = np.asarray(edge_weights, dtype=np.float32)
    lc = np.asarray(label_context, dtype=np.float32)

    lcblk = np.kron(lc.T / 8.0, np.eye(RG, dtype=np.float32)).astype(
        ml_dtypes.bfloat16
    )
    ident = np.eye(P, dtype=np.float32).astype(ml_dtypes.bfloat16)

    in_maps = []
    offs = []
    for core in range(8):
        b, hc = core // 4, core % 4
        uu, ewp, ys, off = _prep_core(u, ew, b, hc)
        offs.append(off)
        in_maps.append({"uu": uu, "ew": ewp, "lcblk": lcblk, "ident": ident})

    kwargs = {}
    if _trace:
        kwargs = dict(trace=True, trace_cores=[0], tmpdir=_tmpdir)
    res = run_bass_kernel_spmd(nc, in_maps, core_ids=list(range(8)), **kwargs)

    out = np.zeros((2, 1, C, 512, 512), dtype=np.float32)
    for core in range(8):
        b, hc = core // 4, core % 4
        yo = np.asarray(res.results[core]["yout"], dtype=np.float32)
        slab = yo.transpose(2, 3, 1, 0).reshape(C, YTP, W)
        off = offs[core]
        out[b, 0, :, 128 * hc : 128 * (hc + 1), :] = slab[
            :, 1 + off : 1 + off + OWN, :
        ]
    if _trace:
        return out, res
    return out


# revision 28
# speedup vs baseline: 1.0124x; 1.0124x over previous
"""MeanField CRF message-passing kernel for 8 Trainium2 NeuronCores.

Sharding: (B=2) x (H into 4 chunks of 128 rows) = 8 slabs, each with a
5-row halo on slab-interior edges (5 mean-field iterations x 1-row
stencil reach), so cores run fully independently (no collectives).

Per-core layouts (all 16-bit except fp32 softmax sums / PSUM):
  x-layout  : [x mod 128 -> partitions, (xblock, class, y) -> free]
  C-packed  : [(class*6+row)=126 (+2 pad) -> partitions, x -> free]
Math per iteration (equivalent-transformed from the reference):
  E   = exp(-Y)                        (ACT, bf16, class-major blocks)
  s   = sum_c E ; r = 1/s              (DVE reduce f32 + recip -> bf16)
  m   = blockdiag(LC^T/8) @ E_C        (PE matmul; E_C via XBAR DMA transpose)
  w2_d= ew_d * shift_d(r)              (DVE bf16 2x, normalizer folded)
  t_d = w2_d * shift_d(m)              (DVE bf16 2x muls on XBAR-transposed m)
  Y   = u + sum_d t_d                  (PE identity-matmuls accumulate in PSUM
                                        with per-dy shifted out-APs; ACT evac)
Window boundary rows (2 per seam) are evacuated as partials and combined
with the next window's PSUM by one tiny DVE add per seam.
Final cost = Y after iteration 5 (no softmax on the last iteration).
"""

import sys

sys.path.insert(0, "/opt/trn_rl_repo")

import numpy as np
import ml_dtypes

import concourse.bass as bass
import concourse.bacc as bacc
import concourse.tile as tile
from concourse import mybir
from concourse.bass_utils import run_bass_kernel_spmd

F32 = mybir.dt.float32
F16 = mybir.dt.float16
BF16 = mybir.dt.bfloat16

P = 128          # partitions
C = 21           # classes
RG = 6           # y-rows per packed group (21*6=126 of 128 partitions)
PK = 128         # padded packed-block size
NB = 23          # row-groups per slab (138 = 6*23)
YT = 138         # slab rows (128 own + 2*5 halo)
YTP = 140        # padded rows (1 pad row each end)
XB = 4           # x blocks (512 = 4*128)
D = 8            # directions
W = 512
HALO = 5
OWN = 128
MAX_ITER = 5
DIRS = [(0, 1), (0, -1), (1, 0), (-1, 0), (1, 1), (1, -1), (-1, 1), (-1, -1)]
# (dy, [(dir index, dx), ...])
GROUPS = [
    (0, [(0, 1), (1, -1)]),
    (1, [(2, 0), (4, 1), (5, -1)]),
    (-1, [(3, 0), (6, 1), (7, -1)]),
]
WINDOWS = [(0, 6), (6, 6), (12, 6), (18, 5)]  # (first group, n groups)
ZN = 38          # PSUM window rows (36 own + 2 boundary)
CA = 13          # class-split sizes (matmul out <= 512 f32 = 1 PSUM bank)
CB = C - CA

_CACHED_NC = None

ADD = mybir.AluOpType.add


def build_nc():
    nc = bacc.Bacc("TRN2")
    uu_d = nc.dram_tensor("uu", [P, XB, C, YTP], BF16, kind="ExternalInput")
    ew_d = nc.dram_tensor("ew", [P, D, XB, YT], F16, kind="ExternalInput")
    lcb_d = nc.dram_tensor("lcblk", [C * RG, C * RG], BF16, kind="ExternalInput")
    ide_d = nc.dram_tensor("ident", [P, P], BF16, kind="ExternalInput")
    yout_d = nc.dram_tensor("yout", [P, XB, C, YTP], BF16, kind="ExternalOutput")

    with tile.TileContext(nc) as tc:
        with (
            tc.tile_pool(name="state", bufs=1) as st,
            tc.tile_pool(name="mxp", bufs=2) as mxp,
            tc.tile_pool(name="tp", bufs=12) as tp,
            tc.tile_pool(name="sbp", bufs=2) as sbp,
            tc.tile_pool(name="trp", bufs=2) as trp,
            tc.tile_pool(name="pm", bufs=2, space="PSUM") as pm,
            tc.tile_pool(name="pw", bufs=3, space="PSUM") as pw,
        ):
            UU = st.tile([P, XB, C, YTP], BF16)
            Y = st.tile([P, XB, C, YTP], BF16)
            EB = st.tile([P, XB, NB, PK], BF16)   # exp(-Y), class-major blocks
            EC = st.tile([P, XB, NB, PK], BF16)   # transposed E (C-packed)
            MC = st.tile([P, PK + NB * W + PK], BF16)  # m, flat, C-packed
            EWs = st.tile([P, D, XB, YT], F16)
            W2P = st.tile([P, D, XB, YTP], BF16)
            S0 = st.tile([P, YT, XB], BF16)
            S0B = st.tile([P, YT, XB], BF16)
            RP = st.tile([P, YT, XB], BF16)
            RM = st.tile([P, YT, XB], BF16)
            LCB = st.tile([C * RG, C * RG], BF16)
            IDE = st.tile([P, P], BF16)
            ZR2 = st.tile([P, C * 2], BF16)

            nc.sync.dma_start(out=UU[:], in_=uu_d[:])
            nc.sync.dma_start(out=EWs[:], in_=ew_d[:])
            nc.sync.dma_start(out=LCB[:], in_=lcb_d[:])
            nc.sync.dma_start(out=IDE[:], in_=ide_d[:])
            nc.gpsimd.memset(Y[:], 0)
            nc.gpsimd.memset(EB[:], 0)
            nc.gpsimd.memset(MC[:], 0)
            nc.gpsimd.memset(W2P[:], 0)
            nc.gpsimd.memset(RP[:], 0)
            nc.gpsimd.memset(RM[:], 0)
            nc.vector.memset(ZR2[:], 0)

            def emit_exp(it, b0, b1):
                srcT = UU if it == 0 else Y
                for xb in range(XB):
                    ev = EB[:, xb, b0:b1, 0:126].rearrange(
                        "p b (c r) -> p c b r", c=C, r=RG
                    )
                    yv = srcT[:, xb, :, 1 + b0 * RG : 1 + b1 * RG].rearrange(
                        "p c (b r) -> p c b r", b=b1 - b0, r=RG
                    )
                    nc.scalar.activation(
                        out=ev, in_=yv,
                        func=mybir.ActivationFunctionType.Exp, scale=-1.0,
                    )

            def emit_tree(b0, b1):
                nb = b1 - b0
                for xb in range(XB):
                    ebv = EB[:, xb, b0:b1, 0:126].rearrange(
                        "p b (c r) -> p b c r", c=C, r=RG
                    )
                    tr = trp.tile([P, NB, 11, RG], BF16, tag="tr")
                    tv = tr[:, b0:b1]
                    nc.vector.tensor_add(
                        out=tv[:, :, 0:10, :], in0=ebv[:, :, 0:10, :],
                        in1=ebv[:, :, 10:20, :],
                    )
                    nc.vector.tensor_add(
                        out=tv[:, :, 0:5, :], in0=tv[:, :, 0:5, :],
                        in1=tv[:, :, 5:10, :],
                    )
                    nc.vector.tensor_add(
                        out=tv[:, :, 0:1, :], in0=tv[:, :, 0:1, :],
                        in1=ebv[:, :, 20:21, :],
                    )
                    nc.vector.tensor_add(
                        out=tv[:, :, 0:2, :], in0=tv[:, :, 0:2, :],
                        in1=tv[:, :, 2:4, :],
                    )
                    nc.vector.tensor_add(
                        out=tv[:, :, 0:1, :], in0=tv[:, :, 0:1, :],
                        in1=tv[:, :, 1:2, :],
                    )
                    nc.vector.tensor_add(
                        out=S0[:, b0 * RG : b1 * RG, xb].rearrange(
                            "p (b r) -> p b r", b=nb, r=RG
                        ),
                        in0=tv[:, :, 0, :],
                        in1=tv[:, :, 4, :],
                    )

            def emit_recip(r0, r1):
                with nc.allow_low_precision(reason="r scales bf16 q"):
                    nc.vector.reciprocal(
                        out=S0B[:, r0:r1, :], in_=S0[:, r0:r1, :]
                    )

            def emit_shift(r0, r1):
                nc.gpsimd.dma_start(
                    out=RP[0 : P - 1, r0:r1, :], in_=S0B[1:P, r0:r1, :]
                )
                nc.gpsimd.dma_start(
                    out=RP[P - 1 : P, r0:r1, 0 : XB - 1],
                    in_=S0B[0:1, r0:r1, 1:XB],
                )
                nc.gpsimd.dma_start(
                    out=RM[1:P, r0:r1, :], in_=S0B[0 : P - 1, r0:r1, :]
                )
                nc.gpsimd.dma_start(
                    out=RM[0:1, r0:r1, 1:XB],
                    in_=S0B[P - 1 : P, r0:r1, 0 : XB - 1],
                )

            def emit_w2(r0, r1):
                for d, (dy, dx) in enumerate(DIRS):
                    rsrc = {-1: RM, 0: S0B, 1: RP}[dx]
                    lo = max(max(0, -dy), r0)
                    hi = min(min(YT, YT - dy), r1)
                    if hi <= lo:
                        continue
                    for xb in range(XB):
                        nc.gpsimd.tensor_mul(
                            out=W2P[:, d, xb, 1 + lo : 1 + hi],
                            in0=EWs[:, d, xb, lo:hi],
                            in1=rsrc[:, lo + dy : hi + dy, xb],
                        )

            def emit_ec(b0, b1):
                for xb in range(XB):
                    nc.sync.dma_start_transpose(
                        out=EC[:, xb, b0:b1, :],
                        in_=EB[:, xb, b0:b1, :].rearrange("p a b -> p (a b)"),
                    )

            BLO = 7            # hoisted prefix: blocks [0,7) -> rows [0,42)
            RLO = BLO * RG
            for it in range(MAX_ITER):
                # ---- phase A head (lo part hoisted into prior iteration) -
                if it == 0:
                    emit_exp(it, 0, BLO)
                    emit_tree(0, BLO)
                    emit_recip(0, RLO)
                    emit_shift(0, RLO)
                    emit_w2(0, RLO - 3)
                    emit_ec(0, BLO)
                emit_exp(it, BLO, NB)
                emit_tree(BLO, NB)
                emit_recip(RLO, YT)
                emit_shift(RLO, YT)
                emit_w2(RLO - 3, YT)
                if it == 0:
                    emit_ec(BLO, NB)

                def emit_mm(rb0, rb1):
                    for rb in range(rb0, rb1):
                        mcp = pm.tile([126, W], F32, tag="mcp")
                        nc.tensor.matmul(
                            out=mcp[:],
                            lhsT=LCB[:],
                            rhs=EC[0:126, :, rb, :],
                            start=True, stop=True,
                        )
                        nc.scalar.copy(
                            out=MC[0:126, PK + rb * W : PK + (rb + 1) * W],
                            in_=mcp[:],
                        )

                emit_mm(0, BLO)
                # pre-emit window 0's shift transposes so they run as soon
                # as the first blocks are evacuated; EC_hi and the remaining
                # matmuls are emitted after (and so execute after) them.
                mxt0 = {}
                for dx in (1, -1, 0):
                    t_mx = mxp.tile([P, 6, XB, PK], BF16, tag=f"mx{dx + 1}")
                    mxt0[dx] = t_mx
                    nc.sync.dma_start_transpose(
                        out=t_mx[:, 0:6, :, :],
                        in_=MC[:, PK + dx : PK + dx + 6 * W],
                    )
                if it > 0:
                    emit_ec(BLO, 13)
                emit_mm(BLO, 13)

                # ---- phase B: XBAR shift transposes, DVE muls, PE accum --
                sbt_prev = {}
                for w, (g0, ng) in enumerate(WINDOWS):
                    zw = g0 * RG            # window base row (pad coords)
                    n = ng * RG             # own rows in window
                    zn = n + 2              # PSUM rows incl. boundary pair
                    last = w == len(WINDOWS) - 1
                    if w == 0:
                        mxt = mxt0
                    else:
                        mxt = {}
                        for dx in (1, -1, 0):
                            t_mx = mxp.tile(
                                [P, 6, XB, PK], BF16, tag=f"mx{dx + 1}"
                            )
                            mxt[dx] = t_mx
                            a0 = PK + g0 * W + dx
                            nc.sync.dma_start_transpose(
                                out=t_mx[:, 0:ng, :, :],
                                in_=MC[:, a0 : a0 + ng * W],
                            )
                    for xb in range(XB):
                        # -- DVE: 8 products t_d = w2_d * m_shift ----------
                        ts = []
                        for dy, dirs_g in GROUPS:
                            z0 = zw - dy + 1
                            for d, dx in dirs_g:
                                t = tp.tile([P, C, 36], BF16, tag="tt")
                                tv = t[:, :, 0 : ng * RG].rearrange(
                                    "p c (g r) -> p g c r", g=ng, r=RG
                                )
                                mxv = mxt[dx][:, 0:ng, xb, 0:126].rearrange(
                                    "p g (c r) -> p g c r", c=C, r=RG
                                )
                                w2v = (
                                    W2P[:, d, xb, z0 : z0 + n]
                                    .rearrange(
                                        "p (g c r) -> p g c r", g=ng, c=1, r=RG
                                    )
                                    .to_broadcast((P, ng, C, RG))
                                )
                                nc.vector.tensor_mul(out=tv, in0=mxv, in1=w2v)
                                ts.append((dy, t))
                        # -- PE: accumulate u + 8 terms in PSUM ------------
                        # matmul out is limited to one PSUM bank (512 f32),
                        # so the [C, ZN] window is split by class range.
                        ywA = pw.tile([P, CA, ZN], F32, tag="ywA")
                        ywB = pw.tile([P, CB, ZN], F32, tag="ywB")
                        halves = ((ywA, 0, CA), (ywB, CA, CB))
                        for yw, c0, cn in halves:
                            if w == 0:
                                nc.tensor.matmul(
                                    out=yw[:, :, 0:zn],
                                    lhsT=IDE[:],
                                    rhs=UU[:, xb, c0 : c0 + cn, zw : zw + zn],
                                    start=True, stop=False,
                                )
                            else:
                                nc.tensor.matmul(
                                    out=yw[:, :, 0:2],
                                    lhsT=IDE[:],
                                    rhs=ZR2[:, 0 : cn * 2].rearrange(
                                        "p (c z) -> p c z", c=cn
                                    ),
                                    start=True, stop=False,
                                )
                                nc.tensor.matmul(
                                    out=yw[:, :, 2:zn],
                                    lhsT=IDE[:],
                                    rhs=UU[:, xb, c0 : c0 + cn, zw + 2 : zw + zn],
                                    start=True, stop=False,
                                )
                            for i, (dy, t) in enumerate(ts):
                                zb = 1 - dy
                                nc.tensor.matmul(
                                    out=yw[:, :, zb : zb + n],
                                    lhsT=IDE[:],
                                    rhs=t[:, c0 : c0 + cn, 0 : ng * RG],
                                    start=False, stop=(i == len(ts) - 1),
                                )
                        # -- evacuate + seam handling ----------------------
                        ea = 0 if w == 0 else 2
                        eb = zn if last else n
                        if not last:
                            sbt = sbp.tile([P, C, 2], F32, tag=f"sb{xb}")
                        for yw, c0, cn in halves:
                            nc.scalar.copy(
                                out=Y[:, xb, c0 : c0 + cn, zw + ea : zw + eb],
                                in_=yw[:, :, ea:eb],
                            )
                            if not last:
                                nc.scalar.copy(
                                    out=sbt[:, c0 : c0 + cn, :],
                                    in_=yw[:, :, n : n + 2],
                                )
                            if w > 0:
                                nc.vector.tensor_add(
                                    out=Y[:, xb, c0 : c0 + cn, zw : zw + 2],
                                    in0=sbt_prev[xb][:, c0 : c0 + cn, :],
                                    in1=yw[:, :, 0:2],
                                )
                        if not last:
                            sbt_prev[xb] = sbt
                    if w == 0:
                        if it > 0:
                            emit_ec(13, NB)
                        emit_mm(13, NB)
                    if w == 1 and it < MAX_ITER - 1:
                        # hoist next iteration's low-row pipeline: rows
                        # [0,42) of Y are final after window 1's evac+seam.
                        emit_exp(it + 1, 0, BLO)
                        emit_tree(0, BLO)
                        emit_recip(0, RLO)
                        emit_shift(0, RLO)
                        emit_w2(0, RLO - 3)
                        emit_ec(0, BLO)

            nc.sync.dma_start(out=yout_d[:], in_=Y[:])

    nc.finalize()
    return nc


def _prep_core(u, ew, b, hc):
    y0 = 128 * hc
    ys = min(max(y0 - HALO, 0), 512 - YT)
    u_slab = u[b, 0, :, ys : ys + YT, :]          # [21, 138, 512]
    ew_slab = ew[b, :, ys : ys + YT, :]           # [8, 138, 512]
    uu = np.zeros((P, XB, C, YTP), np.float32)
    uu[:, :, :, 1 : 1 + YT] = u_slab.reshape(C, YT, XB, P).transpose(3, 2, 0, 1)
    ewp = np.ascontiguousarray(
        ew_slab.reshape(D, YT, XB, P).transpose(3, 0, 2, 1), dtype=np.float16
    )
    return uu.astype(ml_dtypes.bfloat16), ewp, ys, y0 - ys


def kernel(unary, edge_weights, label_context, _trace=False, _tmpdir=None):
    global _CACHED_NC
    if _CACHED_NC is None:
        _CACHED_NC = build_nc()
    nc = _CACHED_NC

    u = np.asarray(unary, dtype=np.float32)
    ew = np.asarray(edge_weights, dtype=np.float32)
    lc = np.asarray(label_context, dtype=np.float32)

    lcblk = np.kron(lc.T / 8.0, np.eye(RG, dtype=np.float32)).astype(
        ml_dtypes.bfloat16
    )
    ident = np.eye(P, dtype=np.float32).astype(ml_dtypes.bfloat16)

    in_maps = []
    offs = []
    for core in range(8):
        b, hc = core // 4, core % 4
        uu, ewp, ys, off = _prep_core(u, ew, b, hc)
        offs.append(off)
        in_maps.append({"uu": uu, "ew": ewp, "lcblk": lcblk, "ident": ident})

    kwargs = {}
    if _trace:
        kwargs = dict(trace=True, trace_cores=[0], tmpdir=_tmpdir)
    res = run_bass_kernel_spmd(nc, in_maps, core_ids=list(range(8)), **kwargs)

    out = np.zeros((2, 1, C, 512, 512), dtype=np.float32)
    for core in range(8):
        b, hc = core // 4, core % 4
        yo = np.asarray(res.results[core]["yout"], dtype=np.float32)
        slab = yo.transpose(2, 3, 1, 0).reshape(C, YTP, W)
        off = offs[core]
        out[b, 0, :, 128 * hc : 128 * (hc + 1), :] = slab[
            :, 1 + off : 1 + off + OWN, :
        ]
    if _trace:
        return out, res
    return out


# revision 29
# speedup vs baseline: 1.0164x; 1.0039x over previous
"""MeanField CRF message-passing kernel for 8 Trainium2 NeuronCores.

Sharding: (B=2) x (H into 4 chunks of 128 rows) = 8 slabs, each with a
5-row halo on slab-interior edges (5 mean-field iterations x 1-row
stencil reach), so cores run fully independently (no collectives).

Per-core layouts (all 16-bit except fp32 softmax sums / PSUM):
  x-layout  : [x mod 128 -> partitions, (xblock, class, y) -> free]
  C-packed  : [(class*6+row)=126 (+2 pad) -> partitions, x -> free]
Math per iteration (equivalent-transformed from the reference):
  E   = exp(-Y)                        (ACT, bf16, class-major blocks)
  s   = sum_c E ; r = 1/s              (DVE bf16 2x add-tree + reciprocal)
  m   = blockdiag(LC^T/8) @ E_C        (PE matmul; E_C via XBAR DMA transpose)
  w2_d= ew_d * shift_d(r)              (gpsimd muls, normalizer folded)
  t_d = w2_d * shift_d(m)              (DVE bf16 2x muls on XBAR-transposed m)
  Y   = u + sum_d t_d                  (PE identity-matmuls accumulate in PSUM
                                        with per-dy shifted out-APs; ACT evac)
Window boundary rows (2 per seam) are evacuated as partials and combined
with the next window's PSUM by one tiny DVE add per seam.  The low-row
prefix of the next iteration (exp/sums/recip/r-shift/w2/E-transpose for
rows < 42) is software-pipelined into window 1 of the current iteration,
hiding most of the iteration-boundary latency.
Final cost = Y after iteration 5 (no softmax on the last iteration).
"""

import sys

sys.path.insert(0, "/opt/trn_rl_repo")

import numpy as np
import ml_dtypes

import concourse.bass as bass
import concourse.bacc as bacc
import concourse.tile as tile
from concourse import mybir
from concourse.bass_utils import run_bass_kernel_spmd

F32 = mybir.dt.float32
F16 = mybir.dt.float16
BF16 = mybir.dt.bfloat16

P = 128          # partitions
C = 21           # classes
RG = 6           # y-rows per packed group (21*6=126 of 128 partitions)
PK = 128         # padded packed-block size
NB = 23          # row-groups per slab (138 = 6*23)
YT = 138         # slab rows (128 own + 2*5 halo)
YTP = 140        # padded rows (1 pad row each end)
XB = 4           # x blocks (512 = 4*128)
D = 8            # directions
W = 512
HALO = 5
OWN = 128
MAX_ITER = 5
DIRS = [(0, 1), (0, -1), (1, 0), (-1, 0), (1, 1), (1, -1), (-1, 1), (-1, -1)]
# (dy, [(dir index, dx), ...])
GROUPS = [
    (0, [(0, 1), (1, -1)]),
    (1, [(2, 0), (4, 1), (5, -1)]),
    (-1, [(3, 0), (6, 1), (7, -1)]),
]
WINDOWS = [(0, 6), (6, 6), (12, 6), (18, 5)]  # (first group, n groups)
ZN = 38          # PSUM window rows (36 own + 2 boundary)
CA = 13          # class-split sizes (matmul out <= 512 f32 = 1 PSUM bank)
CB = C - CA

_CACHED_NC = None

ADD = mybir.AluOpType.add


def build_nc():
    nc = bacc.Bacc("TRN2")
    uu_d = nc.dram_tensor("uu", [P, XB, C, YTP], BF16, kind="ExternalInput")
    ew_d = nc.dram_tensor("ew", [P, D, XB, YT], F16, kind="ExternalInput")
    lcb_d = nc.dram_tensor("lcblk", [C * RG, C * RG], BF16, kind="ExternalInput")
    ide_d = nc.dram_tensor("ident", [P, P], BF16, kind="ExternalInput")
    yout_d = nc.dram_tensor("yout", [P, XB, C, YTP], BF16, kind="ExternalOutput")

    with tile.TileContext(nc) as tc:
        with (
            tc.tile_pool(name="state", bufs=1) as st,
            tc.tile_pool(name="mxp", bufs=2) as mxp,
            tc.tile_pool(name="tp", bufs=12) as tp,
            tc.tile_pool(name="sbp", bufs=2) as sbp,
            tc.tile_pool(name="trp", bufs=2) as trp,
            tc.tile_pool(name="pm", bufs=2, space="PSUM") as pm,
            tc.tile_pool(name="pw", bufs=3, space="PSUM") as pw,
        ):
            UU = st.tile([P, XB, C, YTP], BF16)
            Y = st.tile([P, XB, C, YTP], BF16)
            EB = st.tile([P, XB, NB, PK], BF16)   # exp(-Y), class-major blocks
            EC = st.tile([P, XB, NB, PK], BF16)   # transposed E (C-packed)
            MC = st.tile([P, PK + NB * W + PK], BF16)  # m, flat, C-packed
            EWs = st.tile([P, D, XB, YT], F16)
            W2P = st.tile([P, D, XB, YTP], BF16)
            S0 = st.tile([P, YT, XB], BF16)
            S0B = st.tile([P, YT, XB], BF16)
            RP = st.tile([P, YT, XB], BF16)
            RM = st.tile([P, YT, XB], BF16)
            LCB = st.tile([C * RG, C * RG], BF16)
            IDE = st.tile([P, P], BF16)
            ZR2 = st.tile([P, C * 2], BF16)

            nc.sync.dma_start(out=UU[:], in_=uu_d[:])
            nc.sync.dma_start(out=EWs[:], in_=ew_d[:])
            nc.sync.dma_start(out=LCB[:], in_=lcb_d[:])
            nc.sync.dma_start(out=IDE[:], in_=ide_d[:])
            nc.gpsimd.memset(Y[:], 0)
            nc.gpsimd.memset(EB[:], 0)
            nc.gpsimd.memset(MC[:], 0)
            nc.gpsimd.memset(W2P[:], 0)
            nc.gpsimd.memset(RP[:], 0)
            nc.gpsimd.memset(RM[:], 0)
            nc.vector.memset(ZR2[:], 0)

            def emit_exp(it, b0, b1):
                srcT = UU if it == 0 else Y
                for xb in range(XB):
                    ev = EB[:, xb, b0:b1, 0:126].rearrange(
                        "p b (c r) -> p c b r", c=C, r=RG
                    )
                    yv = srcT[:, xb, :, 1 + b0 * RG : 1 + b1 * RG].rearrange(
                        "p c (b r) -> p c b r", b=b1 - b0, r=RG
                    )
                    nc.scalar.activation(
                        out=ev, in_=yv,
                        func=mybir.ActivationFunctionType.Exp, scale=-1.0,
                    )

            def emit_tree(b0, b1):
                nb = b1 - b0
                for xb in range(XB):
                    ebv = EB[:, xb, b0:b1, 0:126].rearrange(
                        "p b (c r) -> p b c r", c=C, r=RG
                    )
                    tr = trp.tile([P, NB, 11, RG], BF16, tag="tr")
                    tv = tr[:, b0:b1]
                    nc.vector.tensor_add(
                        out=tv[:, :, 0:10, :], in0=ebv[:, :, 0:10, :],
                        in1=ebv[:, :, 10:20, :],
                    )
                    nc.vector.tensor_add(
                        out=tv[:, :, 0:5, :], in0=tv[:, :, 0:5, :],
                        in1=tv[:, :, 5:10, :],
                    )
                    nc.vector.tensor_add(
                        out=tv[:, :, 0:1, :], in0=tv[:, :, 0:1, :],
                        in1=ebv[:, :, 20:21, :],
                    )
                    nc.vector.tensor_add(
                        out=tv[:, :, 0:2, :], in0=tv[:, :, 0:2, :],
                        in1=tv[:, :, 2:4, :],
                    )
                    nc.vector.tensor_add(
                        out=tv[:, :, 0:1, :], in0=tv[:, :, 0:1, :],
                        in1=tv[:, :, 1:2, :],
                    )
                    nc.vector.tensor_add(
                        out=S0[:, b0 * RG : b1 * RG, xb].rearrange(
                            "p (b r) -> p b r", b=nb, r=RG
                        ),
                        in0=tv[:, :, 0, :],
                        in1=tv[:, :, 4, :],
                    )

            def emit_recip(r0, r1):
                with nc.allow_low_precision(reason="r scales bf16 q"):
                    nc.vector.reciprocal(
                        out=S0B[:, r0:r1, :], in_=S0[:, r0:r1, :]
                    )

            def emit_shift(r0, r1):
                nc.gpsimd.dma_start(
                    out=RP[0 : P - 1, r0:r1, :], in_=S0B[1:P, r0:r1, :]
                )
                nc.gpsimd.dma_start(
                    out=RP[P - 1 : P, r0:r1, 0 : XB - 1],
                    in_=S0B[0:1, r0:r1, 1:XB],
                )
                nc.gpsimd.dma_start(
                    out=RM[1:P, r0:r1, :], in_=S0B[0 : P - 1, r0:r1, :]
                )
                nc.gpsimd.dma_start(
                    out=RM[0:1, r0:r1, 1:XB],
                    in_=S0B[P - 1 : P, r0:r1, 0 : XB - 1],
                )

            def emit_w2(r0, r1):
                for d, (dy, dx) in enumerate(DIRS):
                    rsrc = {-1: RM, 0: S0B, 1: RP}[dx]
                    lo = max(max(0, -dy), r0)
                    hi = min(min(YT, YT - dy), r1)
                    if hi <= lo:
                        continue
                    for xb in range(XB):
                        nc.gpsimd.tensor_mul(
                            out=W2P[:, d, xb, 1 + lo : 1 + hi],
                            in0=EWs[:, d, xb, lo:hi],
                            in1=rsrc[:, lo + dy : hi + dy, xb],
                        )

            def emit_ec(b0, b1):
                for xb in range(XB):
                    nc.sync.dma_start_transpose(
                        out=EC[:, xb, b0:b1, :],
                        in_=EB[:, xb, b0:b1, :].rearrange("p a b -> p (a b)"),
                    )

            BLO = 7            # hoisted prefix: blocks [0,7) -> rows [0,42)
            RLO = BLO * RG
            for it in range(MAX_ITER):
                # ---- phase A head (lo part hoisted into prior iteration) -
                if it == 0:
                    emit_exp(it, 0, BLO)
                    emit_tree(0, BLO)
                    emit_recip(0, RLO)
                    emit_shift(0, RLO)
                    emit_w2(0, RLO - 3)
                    emit_ec(0, BLO)
                emit_exp(it, BLO, NB)
                emit_tree(BLO, NB)
                emit_recip(RLO, YT)
                emit_shift(RLO, YT)
                emit_w2(RLO - 3, YT)
                if it == 0:
                    emit_ec(BLO, NB)

                def emit_mm(rb0, rb1):
                    for rb in range(rb0, rb1):
                        mcp = pm.tile([126, W], F32, tag="mcp")
                        nc.tensor.matmul(
                            out=mcp[:],
                            lhsT=LCB[:],
                            rhs=EC[0:126, :, rb, :],
                            start=True, stop=True,
                        )
                        nc.scalar.copy(
                            out=MC[0:126, PK + rb * W : PK + (rb + 1) * W],
                            in_=mcp[:],
                        )

                emit_mm(0, BLO)
                # pre-emit window 0's shift transposes so they run as soon
                # as the first blocks are evacuated; EC_hi and the remaining
                # matmuls are emitted after (and so execute after) them.
                mxt0 = {}
                for dx in (1, -1, 0):
                    t_mx = mxp.tile([P, 6, XB, PK], BF16, tag=f"mx{dx + 1}")
                    mxt0[dx] = t_mx
                    nc.sync.dma_start_transpose(
                        out=t_mx[:, 0:6, :, :],
                        in_=MC[:, PK + dx : PK + dx + 6 * W],
                    )
                if it > 0:
                    emit_ec(BLO, 13)
                emit_mm(BLO, 13)

                # ---- phase B: XBAR shift transposes, DVE muls, PE accum --
                sbt_prev = {}
                for w, (g0, ng) in enumerate(WINDOWS):
                    zw = g0 * RG            # window base row (pad coords)
                    n = ng * RG             # own rows in window
                    zn = n + 2              # PSUM rows incl. boundary pair
                    last = w == len(WINDOWS) - 1
                    if w == 0:
                        mxt = mxt0
                    else:
                        mxt = {}
                        for dx in (1, -1, 0):
                            t_mx = mxp.tile(
                                [P, 6, XB, PK], BF16, tag=f"mx{dx + 1}"
                            )
                            mxt[dx] = t_mx
                            a0 = PK + g0 * W + dx
                            nc.sync.dma_start_transpose(
                                out=t_mx[:, 0:ng, :, :],
                                in_=MC[:, a0 : a0 + ng * W],
                            )
                    for xb in range(XB):
                        # -- DVE: 8 products t_d = w2_d * m_shift ----------
                        ts = []
                        for dy, dirs_g in GROUPS:
                            z0 = zw - dy + 1
                            for d, dx in dirs_g:
                                t = tp.tile([P, C, 36], BF16, tag="tt")
                                tv = t[:, :, 0 : ng * RG].rearrange(
                                    "p c (g r) -> p g c r", g=ng, r=RG
                                )
                                mxv = mxt[dx][:, 0:ng, xb, 0:126].rearrange(
                                    "p g (c r) -> p g c r", c=C, r=RG
                                )
                                w2v = (
                                    W2P[:, d, xb, z0 : z0 + n]
                                    .rearrange(
                                        "p (g c r) -> p g c r", g=ng, c=1, r=RG
                                    )
                                    .to_broadcast((P, ng, C, RG))
                                )
                                nc.vector.tensor_mul(out=tv, in0=mxv, in1=w2v)
                                ts.append((dy, t))
                        # -- PE: accumulate u + 8 terms in PSUM ------------
                        # matmul out is limited to one PSUM bank (512 f32),
                        # so the [C, ZN] window is split by class range.
                        ywA = pw.tile([P, CA, ZN], F32, tag="ywA")
                        ywB = pw.tile([P, CB, ZN], F32, tag="ywB")
                        halves = ((ywA, 0, CA), (ywB, CA, CB))
                        for yw, c0, cn in halves:
                            if w == 0:
                                nc.tensor.matmul(
                                    out=yw[:, :, 0:zn],
                                    lhsT=IDE[:],
                                    rhs=UU[:, xb, c0 : c0 + cn, zw : zw + zn],
                                    start=True, stop=False,
                                )
                            else:
                                nc.tensor.matmul(
                                    out=yw[:, :, 0:2],
                                    lhsT=IDE[:],
                                    rhs=ZR2[:, 0 : cn * 2].rearrange(
                                        "p (c z) -> p c z", c=cn
                                    ),
                                    start=True, stop=False,
                                )
                                nc.tensor.matmul(
                                    out=yw[:, :, 2:zn],
                                    lhsT=IDE[:],
                                    rhs=UU[:, xb, c0 : c0 + cn, zw + 2 : zw + zn],
                                    start=True, stop=False,
                                )
                            for i, (dy, t) in enumerate(ts):
                                zb = 1 - dy
                                nc.tensor.matmul(
                                    out=yw[:, :, zb : zb + n],
                                    lhsT=IDE[:],
                                    rhs=t[:, c0 : c0 + cn, 0 : ng * RG],
                                    start=False, stop=(i == len(ts) - 1),
                                )
                        # -- evacuate + seam handling ----------------------
                        ea = 0 if w == 0 else 2
                        eb = zn if last else n
                        if not last:
                            sbt = sbp.tile([P, C, 2], F32, tag=f"sb{xb}")
                        for yw, c0, cn in halves:
                            nc.scalar.copy(
                                out=Y[:, xb, c0 : c0 + cn, zw + ea : zw + eb],
                                in_=yw[:, :, ea:eb],
                            )
                            if not last:
                                nc.scalar.copy(
                                    out=sbt[:, c0 : c0 + cn, :],
                                    in_=yw[:, :, n : n + 2],
                                )
                            if w > 0:
                                nc.vector.tensor_add(
                                    out=Y[:, xb, c0 : c0 + cn, zw : zw + 2],
                                    in0=sbt_prev[xb][:, c0 : c0 + cn, :],
                                    in1=yw[:, :, 0:2],
                                )
                        if not last:
                            sbt_prev[xb] = sbt
                    if w == 0:
                        if it > 0:
                            emit_ec(13, NB)
                        emit_mm(13, NB)
                    if w == 1 and it < MAX_ITER - 1:
                        # hoist next iteration's low-row pipeline: rows
                        # [0,42) of Y are final after window 1's evac+seam.
                        emit_exp(it + 1, 0, BLO)
                        emit_tree(0, BLO)
                        emit_recip(0, RLO)
                        emit_shift(0, RLO)
                        emit_w2(0, RLO - 3)
                        emit_ec(0, BLO)

            nc.sync.dma_start(out=yout_d[:], in_=Y[:])

    nc.finalize()
    return nc


def _prep_core(u, ew, b, hc):
    y0 = 128 * hc
    ys = min(max(y0 - HALO, 0), 512 - YT)
    u_slab = u[b, 0, :, ys : ys + YT, :]          # [21, 138, 512]
    ew_slab = ew[b, :, ys : ys + YT, :]           # [8, 138, 512]
    uu = np.zeros((P, XB, C, YTP), np.float32)
    uu[:, :, :, 1 : 1 + YT] = u_slab.reshape(C, YT, XB, P).transpose(3, 2, 0, 1)
    ewp = np.ascontiguousarray(
        ew_slab.reshape(D, YT, XB, P).transpose(3, 0, 2, 1), dtype=np.float16
    )
    return uu.astype(ml_dtypes.bfloat16), ewp, ys, y0 - ys


def kernel(unary, edge_weights, label_context, _trace=False, _tmpdir=None):
    global _CACHED_NC
    if _CACHED_NC is None:
        _CACHED_NC = build_nc()
    nc = _CACHED_NC

    u = np.asarray(unary, dtype=np.float32)
    ew = np.asarray(edge_weights, dtype=np.float32)
    lc = np.asarray(label_context, dtype=np.float32)

    lcblk = np.kron(lc.T / 8.0, np.eye(RG, dtype=np.float32)).astype(
        ml_dtypes.bfloat16
    )
    ident = np.eye(P, dtype=np.float32).astype(ml_dtypes.bfloat16)

    in_maps = []
    offs = []
    for core in range(8):
        b, hc = core // 4, core % 4
        uu, ewp, ys, off = _prep_core(u, ew, b, hc)
        offs.append(off)
        in_maps.append({"uu": uu, "ew": ewp, "lcblk": lcblk, "ident": ident})

    kwargs = {}
    if _trace:
        kwargs = dict(trace=True, trace_cores=[0], tmpdir=_tmpdir)
    res = run_bass_kernel_spmd(nc, in_maps, core_ids=list(range(8)), **kwargs)

    out = np.zeros((2, 1, C, 512, 512), dtype=np.float32)
    for core in range(8):
        b, hc = core // 4, core % 4
        yo = np.asarray(res.results[core]["yout"], dtype=np.float32)
        slab = yo.transpose(2, 3, 1, 0).reshape(C, YTP, W)
        off = offs[core]
        out[b, 0, :, 128 * hc : 128 * (hc + 1), :] = slab[
            :, 1 + off : 1 + off + OWN, :
        ]
    if _trace:
        return out, res
    return out
